# revision 27
# baseline (speedup 1.0000x reference)
"""Trainium2 Bass kernel: GQA attention (B=2, S=2048, D=2048, 32 q-heads,
8 kv-heads, head_dim 64, RoPE interleaved, causal) on 8 NeuronCores.

Sharding: tensor-parallel over heads. Core c owns q-heads 4c..4c+3 (= kv head
c) for BOTH batch elements, computes q/k/v projections + RoPE + causal
attention for those heads over all queries, then an 8-core mesh AllToAll
redistributes attention outputs so core c holds all 2048 head-dims for its
(batch, query-quarter) = (c//4, c%4) row block and computes the output
projection locally. The kernel returns out^T row-shards; the host only
transposes/concatenates.

v3 highlights:
  - bf16 operands everywhere on the PE paths (2 elem/cycle streaming); PSUM
    accumulation stays fp32.
  - Softmax normalization deferred to the DESTINATION core: the AllToAll
    carries unnormalized attn outputs + denominator rows (65-row blocks);
    the destination does one batched reciprocal [32,256] + selector-matmul
    broadcast + elementwise scale.
  - Causal masks are applied by the PE (constant triangle / all-NEG tiles
    matmul'd against identity, accumulated into the score PSUM) so the
    Scalar-engine exp stream never waits on the Vector engine.
  - Projection PSUM is drained to SBUF by the otherwise-idle Scalar engine
    so RoPE (DVE) is off the critical path of the next projection block.
  - Output projection runs transposed (tokens on PSUM partitions, output
    dims as the 512-wide moving operand): 4x fewer matmul instructions,
    and the h=0 half overlaps the second AllToAll.
  - Partial (128-key) score blocks processed in pairs so every exp call is
    a full [128,1024] activation. Same-bank PSUM writes always come from
    the same PE row-group (concurrent row-groups to one bank are fatal).
"""

import numpy as np

B, S, D = 2, 2048, 2048
NH, NKV, HD = 32, 8, 64
THETA = 10000.0
NCORES = 8
NEG = -1.0e30

_BUILT = None


def _swap_mask():
    m = []
    for i in range(16):
        m += [2 * i + 1, 2 * i]
    return m


def _build():
    """Build + compile the SPMD Bass program (once per process)."""
    global _BUILT
    if _BUILT is not None:
        return _BUILT

    from contextlib import ExitStack

    import concourse.tile as tile
    from concourse import bacc, mybir

    f32 = mybir.dt.float32
    bf16 = mybir.dt.bfloat16
    AF = mybir.ActivationFunctionType

    nc = bacc.Bacc(
        "TRN2", target_bir_lowering=False, debug=False, num_devices=NCORES
    )

    xT = nc.dram_tensor("xT", [B, 16, 2, 128, 1024], bf16, kind="ExternalInput").ap()
    wqd = nc.dram_tensor("wqd", [128, 4096], bf16, kind="ExternalInput").ap()
    wkvd = nc.dram_tensor("wkvd", [128, 2048], bf16, kind="ExternalInput").ap()
    woT = nc.dram_tensor("woT", [D, D], bf16, kind="ExternalInput").ap()
    cosd = nc.dram_tensor("cosd", [128, B * S], bf16, kind="ExternalInput").ap()
    sind = nc.dram_tensor("sind", [128, B * S], bf16, kind="ExternalInput").ap()
    mTd = nc.dram_tensor("mTd", [128, 256], bf16, kind="ExternalInput").ap()
    onesd = nc.dram_tensor("onesd", [128, 1], bf16, kind="ExternalInput").ap()
    seld = nc.dram_tensor("seld", [32, 2048], bf16, kind="ExternalInput").ap()
    outT = nc.dram_tensor("outT", [512, D], f32, kind="ExternalOutput").ap()

    SW = _swap_mask()
    SCALE = 1.0 / float(np.sqrt(HD))

    with tile.TileContext(nc) as tc, ExitStack() as top:
        top.enter_context(
            nc.allow_low_precision(reason="bf16 matmul/activation inputs by design")
        )
        res = top.enter_context(tc.tile_pool(name="resident", bufs=1))
        # RoPE'd projections, resident across phases
        qt = [res.tile([128, B * S], bf16, tag=f"qt{p}", name=f"qt{p}") for p in range(2)]
        kt = res.tile([128, B * S], bf16, tag="kt")  # kv head, duplicated rows
        vt = [res.tile([128, HD + 1], bf16, tag=f"vt{i}", name=f"vt{i}") for i in range(2 * 16)]
        mT = res.tile([128, 256], bf16, tag="mT")  # [triangleT | all-NEG]
        ident = res.tile([128, 128], bf16, tag="ident")
        ones_t = res.tile([128, 1], bf16, tag="ones")
        sel_t = res.tile([32, 2048], bf16, tag="sel")

        dram = top.enter_context(tc.tile_pool(name="dram", bufs=1, space="DRAM"))
        a2a_in = [dram.tile([8, 260, 256], bf16, tag=f"a2ain{h}", name=f"a2ain{h}") for h in range(2)]
        a2a_out = [dram.tile([8, 260, 256], bf16, tag=f"a2aout{h}", name=f"a2aout{h}") for h in range(2)]

        # ---------------- phase 1: projections + RoPE -------------------
        with ExitStack() as ph1:
            wres = ph1.enter_context(tc.tile_pool(name="wres", bufs=1))
            cos_t = wres.tile([128, B * S], bf16, tag="cos")
            sin_t = wres.tile([128, B * S], bf16, tag="sin")
            scr = wres.tile([1, 1], bf16, tag="scr")
            wq_t = wres.tile([128, 4096], bf16, tag="wq")
            wkv_t = wres.tile([128, 2048], bf16, tag="wkv")

            xp = ph1.enter_context(tc.tile_pool(name="xchunk", bufs=1))
            pp = ph1.enter_context(tc.tile_pool(name="projpsum", bufs=1, space="PSUM"))
            tvp = ph1.enter_context(tc.tile_pool(name="vtpsum", bufs=2, space="PSUM"))
            qcp = ph1.enter_context(tc.tile_pool(name="qcopy", bufs=6))
            rtmp = ph1.enter_context(tc.tile_pool(name="ropetmp", bufs=2))

            # first chunks first: the initial projection matmuls need only
            # wq/wkv and x(b0,d0,h0); everything else streams behind.
            nc.sync.dma_start(out=wq_t[:], in_=wqd[:])
            nc.sync.dma_start(out=wkv_t[:], in_=wkvd[:])

            xd = [None] * 16
            for bh in range(4):
                b, half = bh // 2, bh % 2
                qcs = [1024 * half, 1024 * half + 512]
                cols = [2048 * b + qc for qc in qcs]
                for s in range(2):
                    col = cols[s]
                    # dual-stream projections: each weight/x pair is split into
                    # two K=64 half-contractions on disjoint PE row-groups
                    # writing different PSUM banks -- the halves execute
                    # concurrently on the 32x32 sub-arrays, hiding LDWEIGHTS.
                    pq = [
                        [pp.tile([128, 512], f32, tag=f"q{i}{kk}", name=f"pq{bh}{s}{i}{kk}") for kk in range(2)]
                        for i in range(2)
                    ]
                    pkv = [pp.tile([128, 512], f32, tag=f"kv{kk}", name=f"pkv{bh}{s}{kk}") for kk in range(2)]
                    for d in range(16):
                        if s == 0:
                            xd[d] = xp.tile([128, 1024], bf16, tag=f"xd{d}", name=f"xd{bh}{d}")
                            nc.sync.dma_start(out=xd[d][:], in_=xT[b, d, half])
                        if bh == 0 and s == 0 and d == 8:
                            # everything not needed for the first projections
                            # goes behind the first x chunks in the DMA queues
                            nc.sync.dma_start(out=cos_t[:], in_=cosd[:])
                            nc.sync.dma_start(out=sin_t[:], in_=sind[:])
                            nc.sync.dma_start(out=mT[:], in_=mTd[:])
                            nc.sync.dma_start(out=ones_t[:], in_=onesd[:])
                            nc.sync.dma_start(out=sel_t[:], in_=seld[:])
                            from concourse.masks import make_identity

                            make_identity(nc, ident[:])
                            # preload the exp table while ACT is idle
                            nc.scalar.activation(scr[:], ones_t[0:1, 0:1], AF.Exp)
                        st, sp_ = (d == 0), (d == 15)
                        for i in range(2):
                            c0 = 256 * d + 128 * i
                            for kk in range(2):
                                r = slice(64 * kk, 64 * kk + 64)
                                nc.tensor.matmul(
                                    pq[i][kk][:], wq_t[r, c0:c0 + 128],
                                    xd[d][r, 512 * s:512 * s + 512],
                                    start=st, stop=sp_,
                                )
                        for kk in range(2):
                            r = slice(64 * kk, 64 * kk + 64)
                            nc.tensor.matmul(
                                pkv[kk][:], wkv_t[r, 128 * d:128 * d + 128],
                                xd[d][r, 512 * s:512 * s + 512],
                                start=st, stop=sp_,
                            )

                    # drain + merge the K-halves (DVE), freeing the banks
                    qc = [qcp.tile([128, 512], bf16, tag=f"qc{p}", name=f"qc{bh}{s}{p}") for p in range(2)]
                    kc = qcp.tile([128, 512], bf16, tag="kc", name=f"kc{bh}{s}")
                    hi = [qcp.tile([128, 512], f32, tag=f"hi{t}", name=f"hi{bh}{s}{t}") for t in range(3)]
                    # ACT drains the high-half banks (DVE may read only one
                    # PSUM operand), DVE merges
                    nc.scalar.copy(hi[0][:], pq[0][1][:])
                    nc.scalar.copy(hi[1][:], pq[1][1][:])
                    nc.scalar.copy(hi[2][:], pkv[1][:])
                    for p in range(2):
                        nc.vector.tensor_add(qc[p][:], pq[p][0][:], hi[p][:])
                    nc.vector.tensor_add(kc[:], pkv[0][:], hi[2][:])

                    # v transposes from the kc copy (rows 64:128)
                    for j in range(4):
                        ptv = tvp.tile([128, HD], bf16, tag="tv")
                        nc.tensor.transpose(
                            ptv[:], kc[64:128, 128 * j:128 * (j + 1)], ident[64:128, 64:128]
                        )
                        kb = 4 * (2 * half + s) + j
                        nc.vector.tensor_copy(vt[16 * b + kb][:, 0:HD], ptv[:])
                        nc.sync.dma_start(
                            out=vt[16 * b + kb][:, HD:HD + 1], in_=onesd[:, 0:1]
                        )

                    # RoPE on q head-pairs (all-SBUF bf16 DVE ops)
                    for p in range(2):
                        t1 = rtmp.tile([128, 512], bf16, tag="t1")
                        nc.vector.tensor_mul(t1[:], qc[p][:], cos_t[:, col:col + 512])
                        sw = rtmp.tile([128, 512], bf16, tag="sw")
                        nc.vector.stream_shuffle(sw[:], qc[p][:], SW)
                        t2 = rtmp.tile([128, 512], bf16, tag="t2")
                        nc.vector.tensor_mul(t2[:], sw[:], sin_t[:, col:col + 512])
                        nc.vector.tensor_add(qt[p][:, col:col + 512], t1[:], t2[:])

                    # RoPE on k (kc rows 0:64), then duplicate to rows 64:128
                    t1 = rtmp.tile([128, 512], bf16, tag="t1")
                    nc.vector.tensor_mul(t1[0:64, :], kc[0:64, :], cos_t[0:64, col:col + 512])
                    sw = rtmp.tile([128, 512], bf16, tag="sw")
                    nc.vector.stream_shuffle(sw[0:64, :], kc[0:64, :], SW)
                    t2 = rtmp.tile([128, 512], bf16, tag="t2")
                    nc.vector.tensor_mul(t2[0:64, :], sw[0:64, :], sin_t[0:64, col:col + 512])
                    nc.vector.tensor_add(kt[0:64, col:col + 512], t1[0:64, :], t2[0:64, :])
                    nc.sync.dma_start(
                        out=kt[64:128, col:col + 512], in_=kt[0:64, col:col + 512]
                    )

        # wo streams in during phase 2 (DMA queues are nearly idle there)
        wor = top.enter_context(tc.tile_pool(name="wores", bufs=1))
        wo_t = [wor.tile([128, D], bf16, tag=f"wo{e}", name=f"wo{e}") for e in range(16)]
        for e in range(16):
            nc.sync.dma_start(out=wo_t[e][:], in_=woT[128 * e:128 * (e + 1), :])

        # ---------------- phase 2: causal attention ---------------------
        with ExitStack() as ph2:
            spp = ph2.enter_context(tc.tile_pool(name="scorepsum", bufs=3, space="PSUM"))
            avp = ph2.enter_context(tc.tile_pool(name="avpsum", bufs=1, space="PSUM"))
            esp = ph2.enter_context(tc.tile_pool(name="expsbuf", bufs=4))
            avsp = ph2.enter_context(tc.tile_pool(name="avsbuf", bufs=4))

            qv = [
                qt[p][:].rearrange("p (b h u i) -> p b h u i", b=2, h=2, u=4)
                for p in range(2)
            ]

            def mask_mm(sp, c0, which, last):
                # accumulate the causal mask into score PSUM via constant
                # tiles: triangle for the on-diagonal 128 queries, all-NEG
                # where the whole 128-key block is in the future
                if which == "A":
                    nc.tensor.matmul(
                        sp[:, c0:c0 + 128], mT[:, 0:128], ident[:],
                        start=False, stop=last,
                    )
                else:
                    nc.tensor.matmul(
                        sp[:, c0:c0 + 128], mT[:, 128:256], ident[:],
                        start=False, stop=False,
                    )
                    nc.tensor.matmul(
                        sp[:, c0 + 128:c0 + 256], mT[:, 0:128], ident[:],
                        start=False, stop=last,
                    )

            def do_group(b, u, p):
                av = [
                    avp.tile([HD + 1, 512], f32, tag=f"av{hh}", name=f"av{b}{u}{p}{hh}")
                    for hh in range(2)
                ]
                # units: full 128x1024 score blocks, and pairs of partial
                # (h=1-only) 128-key blocks packed into one 128x1024 tile so
                # every exp call is a full 1024-column activation. u==0 leads
                # with the diagonal fulls so the first av write is full-width.
                if u == 0:
                    units = (
                        [("f", 0), ("f", 1)]
                        + [("pp", (2 + 2 * j, 3 + 2 * j)) for j in range(3)]
                        + [("pp", (8, 9))]
                    )
                else:
                    units = (
                        [("f", kb) for kb in range(2 * u)]
                        + [("pp", (2 * u + 2 + 2 * j, 2 * u + 3 + 2 * j)) for j in range(3)]
                        + [("f", 2 * u), ("f", 2 * u + 1)]
                        + [("pp", (2 * u + 8, 2 * u + 9))]
                    )

                def scores(unit):
                    kind, arg = unit
                    sp = spp.tile([128, 1024], f32, tag="sp", name=f"sp{b}{u}{p}{arg}")
                    if kind == "f":
                        kb = arg
                        kcol = 2048 * b + 128 * kb
                        diag = kb in (2 * u, 2 * u + 1)
                        for hh in range(2):
                            r0 = 64 * hh
                            nc.tensor.matmul(
                                sp[:, 512 * hh:512 * hh + 512],
                                kt[r0:r0 + 64, kcol:kcol + 128],
                                qv[p][r0:r0 + 64, b, :, u, :],
                                start=True, stop=not diag,
                            )
                        if diag:
                            which = "A" if kb == 2 * u else "B"
                            for hh in range(2):
                                mask_mm(sp, 512 * hh, which, last=True)
                    else:
                        kba, kbb = arg
                        diag = kba == 2 * u + 8
                        # bank layout: [a_hh0 | b_hh0 | a_hh1 | b_hh1] so each
                        # PSUM bank is written only by one PE row-group (its
                        # writers serialize); concurrent row-groups into one
                        # bank are fatal on HW
                        for hh in range(2):
                            r0 = 64 * hh
                            for j, kb in enumerate((kba, kbb)):
                                kcol = 2048 * b + 128 * kb
                                nc.tensor.matmul(
                                    sp[:, 512 * hh + 256 * j:512 * hh + 256 * j + 256],
                                    kt[r0:r0 + 64, kcol:kcol + 128],
                                    qv[p][r0:r0 + 64, b, 1, u, :],
                                    start=(j == 0), stop=(j == 1 and not diag),
                                )
                        if diag:
                            for hh in range(2):
                                mask_mm(sp, 512 * hh, "A", last=False)
                                mask_mm(sp, 512 * hh + 256, "B", last=True)
                    return sp

                def expav(unit, sp, first, last):
                    kind, arg = unit
                    ex = esp.tile([128, 1024], bf16, tag="ex", name=f"ex{b}{u}{p}{arg}")
                    nc.scalar.activation(ex[:], sp[:], AF.Exp, scale=SCALE)
                    if kind == "f":
                        kb = arg
                        for hh in range(2):
                            nc.tensor.matmul(
                                av[hh][:, 0:512], vt[16 * b + kb][:],
                                ex[:, 512 * hh:512 * hh + 512],
                                start=first, stop=False,
                            )
                    else:
                        kba, kbb = arg
                        for j, kb in enumerate((kba, kbb)):
                            for hh in range(2):
                                nc.tensor.matmul(
                                    av[hh][:, 256:512], vt[16 * b + kb][:],
                                    ex[:, 512 * hh + 256 * j:512 * hh + 256 * j + 256],
                                    start=(first and j == 0),
                                    stop=(last and j == 1),
                                )

                pipe = []
                for unit in units:
                    sp = scores(unit)
                    pipe.append((unit, sp))
                    if len(pipe) > 2:
                        un, sp_ = pipe.pop(0)
                        expav(un, sp_, first=(un == units[0]), last=(un == units[-1]))
                for un, sp_ in pipe:
                    expav(un, sp_, first=(un == units[0]), last=(un == units[-1]))

                # unnormalized attn outputs + denominator row straight out to
                # the AllToAll staging buffer (bf16)
                for hh in range(2):
                    avs_ = avsp.tile([HD + 1, 512], bf16, tag="avs", name=f"avs{b}{u}{p}{hh}")
                    nc.vector.tensor_copy(avs_[:], av[hh][0:HD + 1, :])
                    r0 = 65 * (2 * p + hh)
                    for hf in range(2):
                        dst = 4 * b + 2 * hf + u // 2
                        nc.sync.dma_start(
                            out=a2a_in[u % 2][dst, r0:r0 + 65, :],
                            in_=avs_[:, 256 * hf:256 * hf + 256],
                        )

            def emit_a2a(h):
                nc.gpsimd.collective_compute(
                    "AllToAll",
                    mybir.AluOpType.bypass,
                    replica_groups=[list(range(8))],
                    ins=[a2a_in[h][:].opt()],
                    outs=[a2a_out[h][:].opt()],
                )

            for u in (0, 2):
                for b in range(B):
                    for p in range(2):
                        do_group(b, u, p)
            emit_a2a(0)
            for u in (1, 3):
                for b in range(B):
                    for p in range(2):
                        do_group(b, u, p)
            emit_a2a(1)

        # ---------------- phase 3: normalize + output projection --------
        with ExitStack() as ph3:
            wrh = ph3.enter_context(tc.tile_pool(name="worh", bufs=1))
            wop = ph3.enter_context(tc.tile_pool(name="wopsum", bufs=1, space="PSUM"))
            bcp = ph3.enter_context(tc.tile_pool(name="bcpsum", bufs=2, space="PSUM"))
            wos = ph3.enter_context(tc.tile_pool(name="wosbuf", bufs=8))

            def load_h(h):
                # denominator rows: row 64 of each 65-row block, all sources
                d_t = wrh.tile([32, 256], bf16, tag=f"dt{h}", name=f"dt{h}")
                for src in range(8):
                    nc.sync.dma_start(
                        out=d_t[4 * src:4 * src + 4, :].rearrange("b (o c) -> b o c", o=1),
                        in_=a2a_out[h][src].rearrange("(blk r) c -> blk r c", blk=4)[:, HD:HD + 1, :],
                    )
                rh_t = [
                    wrh.tile([128, 256], bf16, tag=f"rh{h}{e}", name=f"rh{h}{e}")
                    for e in range(16)
                ]
                for e in range(16):
                    src, q = e // 2, e % 2
                    for j in range(2):
                        r0 = 130 * q + 65 * j
                        nc.sync.dma_start(
                            out=rh_t[e][64 * j:64 * j + 64, :],
                            in_=a2a_out[h][src, r0:r0 + HD, :],
                        )
                return d_t, rh_t

            def norm_h(h, d_t, rh_t):
                invd = wrh.tile([32, 256], bf16, tag=f"invd{h}", name=f"invd{h}")
                nc.vector.reciprocal(invd[:], d_t[:])
                for e in range(16):
                    # broadcast 1/denom over the 64 head-dim rows of each head
                    bcs = bcp.tile([128, 256], f32, tag="bcs", name=f"bcs{h}{e}")
                    nc.tensor.matmul(
                        bcs[:], sel_t[:, 128 * e:128 * e + 128], invd[:],
                        start=True, stop=True,
                    )
                    nc.vector.tensor_mul(rh_t[e][:], rh_t[e][:], bcs[:])

            def mloop_h(h, rh_t):
                # transposed projection: tokens on PSUM partitions, output
                # dims as the 512-wide moving operand -> 4x fewer matmuls
                for tb in range(2):
                    po = [
                        wop.tile([128, 512], f32, tag=f"po{db}", name=f"po{h}{tb}{db}")
                        for db in range(4)
                    ]
                    for e in range(16):
                        lhs = rh_t[e][:, 128 * tb:128 * tb + 128]
                        for db in range(4):
                            nc.tensor.matmul(
                                po[db][:], lhs, wo_t[e][:, 512 * db:512 * db + 512],
                                start=(e == 0), stop=(e == 15),
                            )
                    for db in range(4):
                        os_ = wos.tile([128, 512], f32, tag="os")
                        nc.vector.tensor_copy(os_[:], po[db][:])
                        nc.sync.dma_start(
                            out=outT[256 * h + 128 * tb:256 * h + 128 * tb + 128,
                                     512 * db:512 * db + 512],
                            in_=os_[:],
                        )

            d0, rh0 = load_h(0)
            norm_h(0, d0, rh0)
            # h1 loads queue now (they gate on AllToAll-2 completing) so the
            # h0 output writes behind them can't delay them
            d1, rh1 = load_h(1)
            mloop_h(0, rh0)
            # keep the PE clock warm (HAM) across the AllToAll-2 wait
            norm_h(1, d1, rh1)
            mloop_h(1, rh1)

    nc.compile()
    _BUILT = nc
    return nc


def _host_inputs(x, wq, wk, wv, wo):
    """Per-core input maps (host-side layout prep only, no math on x)."""
    import ml_dtypes

    bf16 = ml_dtypes.bfloat16
    x = np.ascontiguousarray(x, dtype=np.float32)
    xT3 = x.transpose(0, 2, 1)
    xT = np.ascontiguousarray(
        xT3.reshape(B, 16, 128, 2, 1024).transpose(0, 1, 3, 2, 4)
    ).astype(bf16)
    woT = np.ascontiguousarray(np.asarray(wo, np.float32).T).astype(bf16)

    inv = THETA ** (-np.arange(32, dtype=np.float64) / 32.0)
    ang = np.outer(inv, np.arange(S, dtype=np.float64))  # [32, S]
    cos1 = np.cos(ang).astype(np.float32)
    sin1 = np.sin(ang).astype(np.float32)
    pairs = (np.arange(128) % 64) // 2
    signs = np.where(np.arange(128) % 2 == 0, -1.0, 1.0).astype(np.float32)
    cosd = np.ascontiguousarray(np.tile(cos1[pairs], (1, B))).astype(bf16)
    sind = np.ascontiguousarray(np.tile(sin1[pairs] * signs[:, None], (1, B))).astype(bf16)

    # mT: [triangleT | all-NEG]; triangleT[q, k] = NEG where k > q
    q_i = np.arange(128)[:, None]
    k_i = np.arange(128)[None, :]
    tri = np.where(k_i > q_i, NEG, 0.0).astype(np.float32)
    mTd = np.concatenate([tri, np.full((128, 128), NEG, np.float32)], axis=1).astype(bf16)
    onesd = np.ones((128, 1), bf16)

    sel = np.zeros((32, 2048), np.float32)
    for e in range(16):
        r = np.arange(128)
        sel[4 * (e // 2) + 2 * (e % 2) + r // 64, 128 * e + r] = 1.0
    seld = sel.astype(bf16)

    wq = np.asarray(wq, np.float32)
    wk = np.asarray(wk, np.float32)
    wv = np.asarray(wv, np.float32)
    in_maps = []
    for c in range(NCORES):
        wqTc = np.ascontiguousarray(wq[256 * c:256 * (c + 1), :].T)  # [2048, 256]
        # [128, 4096]: cols = 256*d + i, partition = row within d-chunk
        wqd_ = np.ascontiguousarray(
            wqTc.reshape(16, 128, 256).transpose(1, 0, 2).reshape(128, 4096)
        ).astype(bf16)
        wkvTc = np.concatenate(
            [wk[64 * c:64 * (c + 1), :].T, wv[64 * c:64 * (c + 1), :].T], axis=1
        )  # [2048, 128]
        wkvd_ = np.ascontiguousarray(
            wkvTc.reshape(16, 128, 128).transpose(1, 0, 2).reshape(128, 2048)
        ).astype(bf16)
        in_maps.append(
            {
                "xT": xT, "wqd": wqd_, "wkvd": wkvd_, "woT": woT,
                "cosd": cosd, "sind": sind, "mTd": mTd,
                "onesd": onesd, "seld": seld,
            }
        )
    return in_maps


def run(x, wq, wk, wv, wo, trace=False):
    """Build, run on 8 cores, assemble full output. Returns (out, results)."""
    from concourse.bass_utils import run_bass_kernel_spmd

    nc = _build()
    in_maps = _host_inputs(x, wq, wk, wv, wo)
    r = run_bass_kernel_spmd(nc, in_maps, list(range(NCORES)), trace=trace)
    out = np.empty((B, S, D), np.float32)
    for c in range(NCORES):
        b, q = c // 4, c % 4
        out[b, 512 * q:512 * (q + 1), :] = r.results[c]["outT"]
    return out, r


def kernel(x, wq, wk, wv, wo):
    out, _ = run(x, wq, wk, wv, wo, trace=False)
    return out


# revision 28
# speedup vs baseline: 1.0452x; 1.0452x over previous
"""Trainium2 Bass kernel: GQA attention (B=2, S=2048, D=2048, 32 q-heads,
8 kv-heads, head_dim 64, RoPE interleaved, causal) on 8 NeuronCores.

Sharding: tensor-parallel over heads. Core c owns q-heads 4c..4c+3 (= kv head
c) for BOTH batch elements, computes q/k/v projections + RoPE + causal
attention for those heads over all queries, then an 8-core mesh AllToAll
redistributes attention outputs so core c holds all 2048 head-dims for its
(batch, query-quarter) = (c//4, c%4) row block and computes the output
projection locally. The kernel returns out^T row-shards; the host only
transposes/concatenates.

v3 highlights:
  - bf16 operands everywhere on the PE paths (2 elem/cycle streaming); PSUM
    accumulation stays fp32.
  - Softmax normalization deferred to the DESTINATION core: the AllToAll
    carries unnormalized attn outputs + denominator rows (65-row blocks);
    the destination does one batched reciprocal [32,256] + selector-matmul
    broadcast + elementwise scale.
  - Causal masks are applied by the PE (constant triangle / all-NEG tiles
    matmul'd against identity, accumulated into the score PSUM) so the
    Scalar-engine exp stream never waits on the Vector engine.
  - Projection PSUM is drained to SBUF by the otherwise-idle Scalar engine
    so RoPE (DVE) is off the critical path of the next projection block.
  - Output projection runs transposed (tokens on PSUM partitions, output
    dims as the 512-wide moving operand): 4x fewer matmul instructions,
    and the h=0 half overlaps the second AllToAll.
  - Partial (128-key) score blocks processed in pairs so every exp call is
    a full [128,1024] activation. Same-bank PSUM writes always come from
    the same PE row-group (concurrent row-groups to one bank are fatal).
"""

import numpy as np

B, S, D = 2, 2048, 2048
NH, NKV, HD = 32, 8, 64
THETA = 10000.0
NCORES = 8
NEG = -1.0e30

_BUILT = None


def _swap_mask():
    m = []
    for i in range(16):
        m += [2 * i + 1, 2 * i]
    return m


def _build():
    """Build + compile the SPMD Bass program (once per process)."""
    global _BUILT
    if _BUILT is not None:
        return _BUILT

    from contextlib import ExitStack

    import concourse.tile as tile
    from concourse import bacc, mybir

    f32 = mybir.dt.float32
    bf16 = mybir.dt.bfloat16
    AF = mybir.ActivationFunctionType

    nc = bacc.Bacc(
        "TRN2", target_bir_lowering=False, debug=False, num_devices=NCORES
    )

    xT = nc.dram_tensor("xT", [B, 16, 2, 128, 1024], bf16, kind="ExternalInput").ap()
    wqd = nc.dram_tensor("wqd", [128, 4096], bf16, kind="ExternalInput").ap()
    wkvd = nc.dram_tensor("wkvd", [128, 2048], bf16, kind="ExternalInput").ap()
    woT = nc.dram_tensor("woT", [D, D], bf16, kind="ExternalInput").ap()
    cosd = nc.dram_tensor("cosd", [128, B * S], bf16, kind="ExternalInput").ap()
    sind = nc.dram_tensor("sind", [128, B * S], bf16, kind="ExternalInput").ap()
    mTd = nc.dram_tensor("mTd", [128, 256], bf16, kind="ExternalInput").ap()
    onesd = nc.dram_tensor("onesd", [128, 1], bf16, kind="ExternalInput").ap()
    seld = nc.dram_tensor("seld", [32, 2048], bf16, kind="ExternalInput").ap()
    outT = nc.dram_tensor("outT", [512, D], f32, kind="ExternalOutput").ap()

    SW = _swap_mask()
    SCALE = 1.0 / float(np.sqrt(HD))

    with tile.TileContext(nc) as tc, ExitStack() as top:
        top.enter_context(
            nc.allow_low_precision(reason="bf16 matmul/activation inputs by design")
        )
        res = top.enter_context(tc.tile_pool(name="resident", bufs=1))
        # RoPE'd projections, resident across phases
        qt = [res.tile([128, B * S], bf16, tag=f"qt{p}", name=f"qt{p}") for p in range(2)]
        kt = res.tile([128, B * S], bf16, tag="kt")  # kv head, duplicated rows
        vt = [res.tile([128, HD + 1], bf16, tag=f"vt{i}", name=f"vt{i}") for i in range(2 * 16)]
        mT = res.tile([128, 256], bf16, tag="mT")  # [triangleT | all-NEG]
        ident = res.tile([128, 128], bf16, tag="ident")
        ones_t = res.tile([128, 1], bf16, tag="ones")
        sel_t = res.tile([32, 2048], bf16, tag="sel")

        dram = top.enter_context(tc.tile_pool(name="dram", bufs=1, space="DRAM"))
        a2a_in = [dram.tile([8, 260, 256], bf16, tag=f"a2ain{h}", name=f"a2ain{h}") for h in range(2)]
        a2a_out = [dram.tile([8, 260, 256], bf16, tag=f"a2aout{h}", name=f"a2aout{h}") for h in range(2)]

        # ---------------- phase 1: projections + RoPE -------------------
        with ExitStack() as ph1:
            wres = ph1.enter_context(tc.tile_pool(name="wres", bufs=1))
            cos_t = wres.tile([128, B * S], bf16, tag="cos")
            sin_t = wres.tile([128, B * S], bf16, tag="sin")
            scr = wres.tile([1, 1], bf16, tag="scr")
            wq_t = wres.tile([128, 4096], bf16, tag="wq")
            wkv_t = wres.tile([128, 2048], bf16, tag="wkv")

            xp = ph1.enter_context(tc.tile_pool(name="xchunk", bufs=6))
            pp = ph1.enter_context(tc.tile_pool(name="projpsum", bufs=1, space="PSUM"))
            tvp = ph1.enter_context(tc.tile_pool(name="vtpsum", bufs=2, space="PSUM"))
            qcp = ph1.enter_context(tc.tile_pool(name="qcopy", bufs=6))
            rtmp = ph1.enter_context(tc.tile_pool(name="ropetmp", bufs=2))

            # first chunks first: the initial projection matmuls need only
            # wq/wkv and x(b0,d0,h0); everything else streams behind.
            nc.sync.dma_start(out=wq_t[:], in_=wqd[:])
            nc.sync.dma_start(out=wkv_t[:], in_=wkvd[:])

            for bh in range(4):
                b, half = bh // 2, bh % 2
                qcs = [1024 * half, 1024 * half + 512]
                cols = [2048 * b + qc for qc in qcs]
                pq = [
                    [pp.tile([128, 512], f32, tag=f"q{s}{i}", name=f"pq{bh}{s}{i}") for i in range(2)]
                    for s in range(2)
                ]
                pkv = [pp.tile([128, 512], f32, tag=f"kv{s}", name=f"pkv{bh}{s}") for s in range(2)]
                for d in range(16):
                    xt_ = xp.tile([128, 1024], bf16, tag="x")
                    nc.sync.dma_start(out=xt_[:], in_=xT[b, d, half])
                    if bh == 0 and d == 8:
                        # everything not needed for the first projections goes
                        # behind the first x chunks in the DMA queues
                        nc.sync.dma_start(out=cos_t[:], in_=cosd[:])
                        nc.sync.dma_start(out=sin_t[:], in_=sind[:])
                        nc.sync.dma_start(out=mT[:], in_=mTd[:])
                        nc.sync.dma_start(out=ones_t[:], in_=onesd[:])
                        nc.sync.dma_start(out=sel_t[:], in_=seld[:])
                        from concourse.masks import make_identity

                        make_identity(nc, ident[:])
                        # preload the exp activation table while ACT is idle
                        nc.scalar.activation(scr[:], ones_t[0:1, 0:1], AF.Exp)
                    st, sp_ = (d == 0), (d == 15)
                    xs = [xt_[:, 0:512], xt_[:, 512:1024]]
                    # weight-consecutive order: each stationary operand loads
                    # once and serves both token chunks
                    for i in range(2):
                        wslice = wq_t[:, 256 * d + 128 * i:256 * d + 128 * i + 128]
                        for s in range(2):
                            nc.tensor.matmul(pq[s][i][:], wslice, xs[s], start=st, stop=sp_)
                    wslice = wkv_t[:, 128 * d:128 * d + 128]
                    for s in range(2):
                        nc.tensor.matmul(pkv[s][:], wslice, xs[s], start=st, stop=sp_)

                for s in range(2):
                    col = cols[s]
                    # ScalarE drains the projection PSUM so the banks free up
                    # without waiting on the DVE RoPE pipeline
                    qc = [qcp.tile([128, 512], bf16, tag=f"qc{p}", name=f"qc{bh}{s}{p}") for p in range(2)]
                    kc = qcp.tile([128, 512], bf16, tag="kc", name=f"kc{bh}{s}")
                    nc.scalar.copy(kc[:], pkv[s][:])
                    for p in range(2):
                        nc.scalar.copy(qc[p][:], pq[s][p][:])

                    # v transposes from the kc copy (rows 64:128)
                    for j in range(4):
                        ptv = tvp.tile([128, HD], bf16, tag="tv")
                        nc.tensor.transpose(
                            ptv[:], kc[64:128, 128 * j:128 * (j + 1)], ident[64:128, 64:128]
                        )
                        kb = 4 * (2 * half + s) + j
                        nc.vector.tensor_copy(vt[16 * b + kb][:, 0:HD], ptv[:])
                        nc.sync.dma_start(
                            out=vt[16 * b + kb][:, HD:HD + 1], in_=onesd[:, 0:1]
                        )

                    # RoPE on q head-pairs (all-SBUF bf16 DVE ops)
                    for p in range(2):
                        t1 = rtmp.tile([128, 512], bf16, tag="t1")
                        nc.vector.tensor_mul(t1[:], qc[p][:], cos_t[:, col:col + 512])
                        sw = rtmp.tile([128, 512], bf16, tag="sw")
                        nc.vector.stream_shuffle(sw[:], qc[p][:], SW)
                        t2 = rtmp.tile([128, 512], bf16, tag="t2")
                        nc.vector.tensor_mul(t2[:], sw[:], sin_t[:, col:col + 512])
                        nc.vector.tensor_add(qt[p][:, col:col + 512], t1[:], t2[:])

                    # RoPE on k (kc rows 0:64), then duplicate to rows 64:128
                    t1 = rtmp.tile([128, 512], bf16, tag="t1")
                    nc.vector.tensor_mul(t1[0:64, :], kc[0:64, :], cos_t[0:64, col:col + 512])
                    sw = rtmp.tile([128, 512], bf16, tag="sw")
                    nc.vector.stream_shuffle(sw[0:64, :], kc[0:64, :], SW)
                    t2 = rtmp.tile([128, 512], bf16, tag="t2")
                    nc.vector.tensor_mul(t2[0:64, :], sw[0:64, :], sin_t[0:64, col:col + 512])
                    nc.vector.tensor_add(kt[0:64, col:col + 512], t1[0:64, :], t2[0:64, :])
                    nc.sync.dma_start(
                        out=kt[64:128, col:col + 512], in_=kt[0:64, col:col + 512]
                    )

        # wo streams in during phase 2 (DMA queues are nearly idle there)
        wor = top.enter_context(tc.tile_pool(name="wores", bufs=1))
        wo_t = [wor.tile([128, D], bf16, tag=f"wo{e}", name=f"wo{e}") for e in range(16)]
        for e in range(16):
            nc.sync.dma_start(out=wo_t[e][:], in_=woT[128 * e:128 * (e + 1), :])

        # ---------------- phase 2: causal attention ---------------------
        with ExitStack() as ph2:
            spp = ph2.enter_context(tc.tile_pool(name="scorepsum", bufs=3, space="PSUM"))
            avp = ph2.enter_context(tc.tile_pool(name="avpsum", bufs=1, space="PSUM"))
            esp = ph2.enter_context(tc.tile_pool(name="expsbuf", bufs=4))
            avsp = ph2.enter_context(tc.tile_pool(name="avsbuf", bufs=4))

            qv = [
                qt[p][:].rearrange("p (b h u i) -> p b h u i", b=2, h=2, u=4)
                for p in range(2)
            ]

            def mask_mm(sp, c0, which, last):
                # accumulate the causal mask into score PSUM via constant
                # tiles: triangle for the on-diagonal 128 queries, all-NEG
                # where the whole 128-key block is in the future
                if which == "A":
                    nc.tensor.matmul(
                        sp[:, c0:c0 + 128], mT[:, 0:128], ident[:],
                        start=False, stop=last,
                    )
                else:
                    nc.tensor.matmul(
                        sp[:, c0:c0 + 128], mT[:, 128:256], ident[:],
                        start=False, stop=False,
                    )
                    nc.tensor.matmul(
                        sp[:, c0 + 128:c0 + 256], mT[:, 0:128], ident[:],
                        start=False, stop=last,
                    )

            def do_group(b, u, p):
                av = [
                    avp.tile([HD + 1, 512], f32, tag=f"av{hh}", name=f"av{b}{u}{p}{hh}")
                    for hh in range(2)
                ]
                # units: full 128x1024 score blocks, and pairs of partial
                # (h=1-only) 128-key blocks packed into one 128x1024 tile so
                # every exp call is a full 1024-column activation. u==0 leads
                # with the diagonal fulls so the first av write is full-width.
                if u == 0:
                    units = (
                        [("f", 0), ("f", 1)]
                        + [("pp", (2 + 2 * j, 3 + 2 * j)) for j in range(3)]
                        + [("pp", (8, 9))]
                    )
                else:
                    units = (
                        [("f", kb) for kb in range(2 * u)]
                        + [("pp", (2 * u + 2 + 2 * j, 2 * u + 3 + 2 * j)) for j in range(3)]
                        + [("f", 2 * u), ("f", 2 * u + 1)]
                        + [("pp", (2 * u + 8, 2 * u + 9))]
                    )

                def scores(unit):
                    kind, arg = unit
                    sp = spp.tile([128, 1024], f32, tag="sp", name=f"sp{b}{u}{p}{arg}")
                    if kind == "f":
                        kb = arg
                        kcol = 2048 * b + 128 * kb
                        diag = kb in (2 * u, 2 * u + 1)
                        for hh in range(2):
                            r0 = 64 * hh
                            nc.tensor.matmul(
                                sp[:, 512 * hh:512 * hh + 512],
                                kt[r0:r0 + 64, kcol:kcol + 128],
                                qv[p][r0:r0 + 64, b, :, u, :],
                                start=True, stop=not diag,
                            )
                        if diag:
                            which = "A" if kb == 2 * u else "B"
                            for hh in range(2):
                                mask_mm(sp, 512 * hh, which, last=True)
                    else:
                        kba, kbb = arg
                        diag = kba == 2 * u + 8
                        # bank layout: [a_hh0 | b_hh0 | a_hh1 | b_hh1] so each
                        # PSUM bank is written only by one PE row-group (its
                        # writers serialize); concurrent row-groups into one
                        # bank are fatal on HW
                        for hh in range(2):
                            r0 = 64 * hh
                            for j, kb in enumerate((kba, kbb)):
                                kcol = 2048 * b + 128 * kb
                                nc.tensor.matmul(
                                    sp[:, 512 * hh + 256 * j:512 * hh + 256 * j + 256],
                                    kt[r0:r0 + 64, kcol:kcol + 128],
                                    qv[p][r0:r0 + 64, b, 1, u, :],
                                    start=(j == 0), stop=(j == 1 and not diag),
                                )
                        if diag:
                            for hh in range(2):
                                mask_mm(sp, 512 * hh, "A", last=False)
                                mask_mm(sp, 512 * hh + 256, "B", last=True)
                    return sp

                def expav(unit, sp, first, last):
                    kind, arg = unit
                    ex = esp.tile([128, 1024], bf16, tag="ex", name=f"ex{b}{u}{p}{arg}")
                    nc.scalar.activation(ex[:], sp[:], AF.Exp, scale=SCALE)
                    if kind == "f":
                        kb = arg
                        for hh in range(2):
                            nc.tensor.matmul(
                                av[hh][:, 0:512], vt[16 * b + kb][:],
                                ex[:, 512 * hh:512 * hh + 512],
                                start=first, stop=False,
                            )
                    else:
                        kba, kbb = arg
                        for j, kb in enumerate((kba, kbb)):
                            for hh in range(2):
                                nc.tensor.matmul(
                                    av[hh][:, 256:512], vt[16 * b + kb][:],
                                    ex[:, 512 * hh + 256 * j:512 * hh + 256 * j + 256],
                                    start=(first and j == 0),
                                    stop=(last and j == 1),
                                )

                pipe = []
                for unit in units:
                    sp = scores(unit)
                    pipe.append((unit, sp))
                    if len(pipe) > 2:
                        un, sp_ = pipe.pop(0)
                        expav(un, sp_, first=(un == units[0]), last=(un == units[-1]))
                for un, sp_ in pipe:
                    expav(un, sp_, first=(un == units[0]), last=(un == units[-1]))

                # unnormalized attn outputs + denominator row straight out to
                # the AllToAll staging buffer (bf16)
                for hh in range(2):
                    avs_ = avsp.tile([HD + 1, 512], bf16, tag="avs", name=f"avs{b}{u}{p}{hh}")
                    nc.vector.tensor_copy(avs_[:], av[hh][0:HD + 1, :])
                    r0 = 65 * (2 * p + hh)
                    for hf in range(2):
                        dst = 4 * b + 2 * hf + u // 2
                        nc.sync.dma_start(
                            out=a2a_in[u % 2][dst, r0:r0 + 65, :],
                            in_=avs_[:, 256 * hf:256 * hf + 256],
                        )

            def emit_a2a(h):
                nc.gpsimd.collective_compute(
                    "AllToAll",
                    mybir.AluOpType.bypass,
                    replica_groups=[list(range(8))],
                    ins=[a2a_in[h][:].opt()],
                    outs=[a2a_out[h][:].opt()],
                )

            for u in (0, 2):
                for b in range(B):
                    for p in range(2):
                        do_group(b, u, p)
            emit_a2a(0)
            for u in (1, 3):
                for b in range(B):
                    for p in range(2):
                        do_group(b, u, p)
            emit_a2a(1)

        # ---------------- phase 3: normalize + output projection --------
        with ExitStack() as ph3:
            wrh = ph3.enter_context(tc.tile_pool(name="worh", bufs=1))
            wop = ph3.enter_context(tc.tile_pool(name="wopsum", bufs=1, space="PSUM"))
            bcp = ph3.enter_context(tc.tile_pool(name="bcpsum", bufs=2, space="PSUM"))
            wos = ph3.enter_context(tc.tile_pool(name="wosbuf", bufs=8))

            def load_h(h):
                # denominator rows: row 64 of each 65-row block, all sources
                d_t = wrh.tile([32, 256], bf16, tag=f"dt{h}", name=f"dt{h}")
                for src in range(8):
                    nc.sync.dma_start(
                        out=d_t[4 * src:4 * src + 4, :].rearrange("b (o c) -> b o c", o=1),
                        in_=a2a_out[h][src].rearrange("(blk r) c -> blk r c", blk=4)[:, HD:HD + 1, :],
                    )
                rh_t = [
                    wrh.tile([128, 256], bf16, tag=f"rh{h}{e}", name=f"rh{h}{e}")
                    for e in range(16)
                ]
                for e in range(16):
                    src, q = e // 2, e % 2
                    for j in range(2):
                        r0 = 130 * q + 65 * j
                        nc.sync.dma_start(
                            out=rh_t[e][64 * j:64 * j + 64, :],
                            in_=a2a_out[h][src, r0:r0 + HD, :],
                        )
                return d_t, rh_t

            def norm_h(h, d_t, rh_t):
                invd = wrh.tile([32, 256], bf16, tag=f"invd{h}", name=f"invd{h}")
                nc.vector.reciprocal(invd[:], d_t[:])
                for e in range(16):
                    # broadcast 1/denom over the 64 head-dim rows of each head
                    bcs = bcp.tile([128, 256], f32, tag="bcs", name=f"bcs{h}{e}")
                    nc.tensor.matmul(
                        bcs[:], sel_t[:, 128 * e:128 * e + 128], invd[:],
                        start=True, stop=True,
                    )
                    nc.vector.tensor_mul(rh_t[e][:], rh_t[e][:], bcs[:])

            def mloop_h(h, rh_t):
                # transposed projection: tokens on PSUM partitions, output
                # dims as the 512-wide moving operand -> 4x fewer matmuls
                for tb in range(2):
                    po = [
                        wop.tile([128, 512], f32, tag=f"po{db}", name=f"po{h}{tb}{db}")
                        for db in range(4)
                    ]
                    for e in range(16):
                        lhs = rh_t[e][:, 128 * tb:128 * tb + 128]
                        for db in range(4):
                            nc.tensor.matmul(
                                po[db][:], lhs, wo_t[e][:, 512 * db:512 * db + 512],
                                start=(e == 0), stop=(e == 15),
                            )
                    for db in range(4):
                        os_ = wos.tile([128, 512], f32, tag="os")
                        nc.vector.tensor_copy(os_[:], po[db][:])
                        nc.sync.dma_start(
                            out=outT[256 * h + 128 * tb:256 * h + 128 * tb + 128,
                                     512 * db:512 * db + 512],
                            in_=os_[:],
                        )

            d0, rh0 = load_h(0)
            norm_h(0, d0, rh0)
            # h1 loads queue now (they gate on AllToAll-2 completing) so the
            # h0 output writes behind them can't delay them
            d1, rh1 = load_h(1)
            mloop_h(0, rh0)
            # keep the PE clock warm (HAM) across the AllToAll-2 wait
            norm_h(1, d1, rh1)
            mloop_h(1, rh1)

    nc.compile()
    _BUILT = nc
    return nc


def _host_inputs(x, wq, wk, wv, wo):
    """Per-core input maps (host-side layout prep only, no math on x)."""
    import ml_dtypes

    bf16 = ml_dtypes.bfloat16
    x = np.ascontiguousarray(x, dtype=np.float32)
    xT3 = x.transpose(0, 2, 1)
    xT = np.ascontiguousarray(
        xT3.reshape(B, 16, 128, 2, 1024).transpose(0, 1, 3, 2, 4)
    ).astype(bf16)
    woT = np.ascontiguousarray(np.asarray(wo, np.float32).T).astype(bf16)

    inv = THETA ** (-np.arange(32, dtype=np.float64) / 32.0)
    ang = np.outer(inv, np.arange(S, dtype=np.float64))  # [32, S]
    cos1 = np.cos(ang).astype(np.float32)
    sin1 = np.sin(ang).astype(np.float32)
    pairs = (np.arange(128) % 64) // 2
    signs = np.where(np.arange(128) % 2 == 0, -1.0, 1.0).astype(np.float32)
    cosd = np.ascontiguousarray(np.tile(cos1[pairs], (1, B))).astype(bf16)
    sind = np.ascontiguousarray(np.tile(sin1[pairs] * signs[:, None], (1, B))).astype(bf16)

    # mT: [triangleT | all-NEG]; triangleT[q, k] = NEG where k > q
    q_i = np.arange(128)[:, None]
    k_i = np.arange(128)[None, :]
    tri = np.where(k_i > q_i, NEG, 0.0).astype(np.float32)
    mTd = np.concatenate([tri, np.full((128, 128), NEG, np.float32)], axis=1).astype(bf16)
    onesd = np.ones((128, 1), bf16)

    sel = np.zeros((32, 2048), np.float32)
    for e in range(16):
        r = np.arange(128)
        sel[4 * (e // 2) + 2 * (e % 2) + r // 64, 128 * e + r] = 1.0
    seld = sel.astype(bf16)

    wq = np.asarray(wq, np.float32)
    wk = np.asarray(wk, np.float32)
    wv = np.asarray(wv, np.float32)
    in_maps = []
    for c in range(NCORES):
        wqTc = np.ascontiguousarray(wq[256 * c:256 * (c + 1), :].T)  # [2048, 256]
        # [128, 4096]: cols = 256*d + i, partition = row within d-chunk
        wqd_ = np.ascontiguousarray(
            wqTc.reshape(16, 128, 256).transpose(1, 0, 2).reshape(128, 4096)
        ).astype(bf16)
        wkvTc = np.concatenate(
            [wk[64 * c:64 * (c + 1), :].T, wv[64 * c:64 * (c + 1), :].T], axis=1
        )  # [2048, 128]
        wkvd_ = np.ascontiguousarray(
            wkvTc.reshape(16, 128, 128).transpose(1, 0, 2).reshape(128, 2048)
        ).astype(bf16)
        in_maps.append(
            {
                "xT": xT, "wqd": wqd_, "wkvd": wkvd_, "woT": woT,
                "cosd": cosd, "sind": sind, "mTd": mTd,
                "onesd": onesd, "seld": seld,
            }
        )
    return in_maps


def run(x, wq, wk, wv, wo, trace=False):
    """Build, run on 8 cores, assemble full output. Returns (out, results)."""
    from concourse.bass_utils import run_bass_kernel_spmd

    nc = _build()
    in_maps = _host_inputs(x, wq, wk, wv, wo)
    r = run_bass_kernel_spmd(nc, in_maps, list(range(NCORES)), trace=trace)
    out = np.empty((B, S, D), np.float32)
    for c in range(NCORES):
        b, q = c // 4, c % 4
        out[b, 512 * q:512 * (q + 1), :] = r.results[c]["outT"]
    return out, r


def kernel(x, wq, wk, wv, wo):
    out, _ = run(x, wq, wk, wv, wo, trace=False)
    return out
